# revision 2
# baseline (speedup 1.0000x reference)
"""MoE routing kernel (nn_DecFCSwitch) for 8 Trainium2 NeuronCores.

Reference computes all 16 expert branches per token then selects one; only
the selected branch matters, so:

  host:   sort tokens by expert, pad each expert's tokens to capacity C,
          relu(x), cast to fp8e4m3 (weights pre-scaled by 256 so values sit
          in e4m3's normal range), lay tensors out partition-major with the
          token matrix expert-major so each expert's tokens are one
          contiguous DMA.  Both biases fold into the host-side gather
          (b' = W_out @ b_in + b_out, exact in f32).
  device: expert-parallel SPMD - core i owns experts {2i, 2i+1}.  Both
          layers are fp8 DoubleRow matmuls (256-deep contraction, 0.5
          cycles/row).  The schedule below (load order/issuers, eviction
          engine per PSUM pair, PSUM slot routing, store splits) was found
          by randomized search against the calibrated TimelineSim cost
          model, restricted to hardware-legal engines (GPSIMD cannot read
          PSUM, so evictions ride ACT/DVE only), and validated bit-stable
          on silicon.  h0 loads via Pool SWDGE (off the HWDGE queue); the
          wz warm-up operand memsets on DVE so Pool's descriptor generation
          starts immediately.  Warm-up matmuls hold the PE p-state so real
          matmuls run at full clock.
  host:   decode fp8 -> f32, scatter rows back to token order,
          out = x + sel + b'.
"""

import os
import sys

import numpy as np

for _p in ("/opt/trn_rl_repo", "/root/.axon_site/_ro/trn_rl_repo"):
    if os.path.isdir(_p) and _p not in sys.path:
        sys.path.insert(0, _p)

import ml_dtypes

B, D, S, NB = 4096, 1024, 256, 16
NCORES = 8
EPC = NB // NCORES
KD = D // 128  # 8
KS = S // 128  # 2
K2 = D // 256  # 4

F8 = ml_dtypes.float8_e4m3
WSCALE = 256.0

_programs = {}
LAST_RESULT = None

# Schedule found by randomized search over TimelineSim (13223 ns).
# loads: (chunk, issuing engine); h0 rides Pool SWDGE, rest SP HWDGE.
# l1ev:  expert -> [(engine, s-tiles)] evictions; engine 0=ACT 1=DVE.
# l2ev:  (expert, low bank, engine, mode) per PSUM pair, in emission order.
# l2ps1: L2 pairs routed through the (freed) L1 PSUM banks.
# stores: (expert, m0, m1, issuer) emitted once banks m0..m1-1 evicted.
PLAN = dict(
    loads=[
        ("w1e0", "sp"),
        ("h0", "pool"),
        ("w1e1", "sp"),
        ("w2e0", "sp"),
        ("h1", "sp"),
        ("w2e1", "sp"),
    ],
    warm=4,
    l1ev={0: [(1, (0, 1))], 1: [(0, (0, 1))]},
    l2ev=[
        (0, 0, 0, "pair"),
        (0, 2, 1, "pair"),
        (0, 4, 1, "pair"),
        (0, 6, 1, "pair"),
        (1, 0, 1, "pair"),
        (1, 2, 0, "pair"),
        (1, 4, 1, "pair"),
        (1, 6, 0, "pair"),
    ],
    l2ord=[(0, 0), (0, 1), (0, 2), (0, 3), (1, 0), (1, 1), (1, 2), (1, 3)],
    l2ps1=[(1, 6)],
    ps1=1,
    ps2=3,
    stores=[(0, 0, 8, "sp"), (1, 0, 4, "sp"), (1, 4, 8, "sp")],
)


def _build_program(C, plan=PLAN):
    import concourse.mybir as mybir
    import concourse.tile as tile
    from concourse import bacc

    f8 = mybir.dt.float8e4
    bf16 = mybir.dt.bfloat16
    f32 = mybir.dt.float32
    copy_f = mybir.ActivationFunctionType.Copy
    DR = mybir.MatmulPerfMode.DoubleRow

    nc = bacc.Bacc()
    hT = nc.declare_dram_parameter("hT", [128, EPC * KD * C], f8, isOutput=False)
    w1 = nc.declare_dram_parameter("w1", [128, EPC * 2048], f8, isOutput=False)
    w2 = nc.declare_dram_parameter("w2", [128, EPC * 2048], f8, isOutput=False)
    yT = nc.declare_dram_parameter("yT", [128, EPC * KD * C], f8, isOutput=True)

    with tile.TileContext(nc) as tc:
        with (
            tc.tile_pool(name="h", bufs=1) as h_pool,
            tc.tile_pool(name="w1p", bufs=1) as w1_pool,
            tc.tile_pool(name="w2p", bufs=1) as w2_pool,
            tc.tile_pool(name="hid", bufs=2) as hid_pool,
            tc.tile_pool(name="yout", bufs=2) as y_pool,
            tc.tile_pool(name="warm", bufs=1) as warm_pool,
            tc.tile_pool(name="ps1", bufs=plan.get("ps1", 1), space="PSUM") as ps1_pool,
            tc.tile_pool(name="ps2", bufs=plan.get("ps2", 3), space="PSUM") as ps2_pool,
        ):
            wz = warm_pool.tile([128, 512], bf16, tag="wz")
            ht = h_pool.tile([128, EPC * KD * C], f8, tag="h")
            w1t = w1_pool.tile([128, EPC * 2048], f8, tag="w1")
            w2t = w2_pool.tile([128, EPC * 2048], f8, tag="w2")
            hid = [
                hid_pool.tile([128, KS * C], f8, tag=f"hid{e}", name=f"hid{e}")
                for e in range(EPC)
            ]
            ybig = [
                y_pool.tile([128, KD * C], f8, tag=f"y{e}", name=f"y{e}")
                for e in range(EPC)
            ]

            # wz on DVE so Pool's SWDGE desc-gen for h0 starts at t~0
            nc.vector.memset(wz[:], 0)

            EC = KD * C
            issuers = {
                "sp": nc.sync,
                "act": nc.scalar,
                "dve": nc.vector,
                "pool": nc.gpsimd,
            }
            load_chunks = {
                "h0": (ht, hT, 0, EC),
                "h1": (ht, hT, EC, 2 * EC),
                "w1e0": (w1t, w1, 0, 2048),
                "w1e1": (w1t, w1, 2048, 4096),
                "w2e0": (w2t, w2, 0, 2048),
                "w2e1": (w2t, w2, 2048, 4096),
            }
            for nm, issuer in plan["loads"]:
                dst, src, c0, c1 = load_chunks[nm]
                issuers[issuer].dma_start(out=dst[:, c0:c1], in_=src[:, c0:c1])

            ht_v = ht[:].rearrange("p (e k c) -> p e k c", e=EPC, k=KD)

            def warm_mm(pp, n):
                for _ in range(n):
                    nc.tensor.matmul(
                        pp[:, 0:512], lhsT=wz[:, 0:128], rhs=wz[:],
                        start=True, stop=True,
                    )

            def l1_matmul(e, t, k2, out_ap):
                base = ((e * KS + t) * K2 + k2) * 256
                nc.tensor.matmul(
                    out_ap,
                    lhsT=w1t[:, base : base + 256].rearrange("p (i m) -> p i m", i=2),
                    rhs=ht_v[:, e, 2 * k2 : 2 * k2 + 2, :],
                    start=(k2 == 0),
                    stop=(k2 == K2 - 1),
                    perf_mode=DR,
                )

            def l2_matmul(e, m, out_ap, hv):
                base = (e * KD + m) * 256
                nc.tensor.matmul(
                    out_ap,
                    lhsT=w2t[:, base : base + 256].rearrange("p (i m) -> p i m", i=2),
                    rhs=hv,
                    start=True,
                    stop=True,
                    perf_mode=DR,
                )

            def evict(eng, dst, src):
                # out = psum/WSCALE cast to fp8; ACT and DVE are the only
                # PSUM-capable eviction engines (GPSIMD cannot read PSUM)
                if eng == 0:
                    nc.scalar.activation(dst, src, copy_f, scale=1.0 / WSCALE)
                else:
                    nc.vector.tensor_scalar_mul(dst, src, 1.0 / WSCALE)

            def store(e, m0, m1, issuer):
                issuers[issuer].dma_start(
                    out=yT[:, (e * KD + m0) * C : (e * KD + m1) * C],
                    in_=ybig[e][:, m0 * C : m1 * C],
                )

            # --- L1 ------------------------------------------------------
            for e in range(EPC):
                pp = ps1_pool.tile([128, 1024], f32, name="pp1")
                if e == 0:
                    warm_mm(pp, plan["warm"])
                for k2 in range(K2):
                    for t in range(KS):
                        l1_matmul(e, t, k2, pp[:, t * 512 : t * 512 + C])
                for eng, ts in plan["l1ev"][e]:
                    if len(ts) == 2:
                        evict(
                            eng,
                            hid[e][:].rearrange("p (t c) -> p t c", t=2),
                            pp[:].rearrange("p (t x) -> p t x", t=2)[:, :, 0:C],
                        )
                    else:
                        (t,) = ts
                        evict(
                            eng,
                            hid[e][:, t * C : (t + 1) * C],
                            pp[:, t * 512 : t * 512 + C],
                        )

            # --- L2 ------------------------------------------------------
            hid_v = {
                e: hid[e][:].rearrange("p (i c) -> p i c", i=KS) for e in range(EPC)
            }
            mm_done = set()
            pp2 = {}

            def evict_l2(ee, lo, eng, mode):
                ppx = pp2[(ee, lo)]
                if mode == "pair":
                    evict(
                        eng,
                        ybig[ee][:, lo * C : (lo + 2) * C].rearrange(
                            "p (t c) -> p t c", t=2
                        ),
                        ppx[:].rearrange("p (t x) -> p t x", t=2)[:, :, 0:C],
                    )
                else:  # split2: two singles on eng and its partner
                    for k, en in ((0, eng), (1, 1 - eng)):
                        evict(
                            en,
                            ybig[ee][:, (lo + k) * C : (lo + k + 1) * C],
                            ppx[:, k * 512 : k * 512 + C],
                        )

            pending_stores = list(plan["stores"])
            banks_done = {0: set(), 1: set()}

            def emit_ready_stores():
                # a store is emitted only once every bank it reads has had
                # its eviction emitted (program order = read-after-write)
                nonlocal pending_stores
                rest = []
                for e_, m0, m1, issuer in pending_stores:
                    if all(m in banks_done[e_] for m in range(m0, m1)):
                        store(e_, m0, m1, issuer)
                    else:
                        rest.append((e_, m0, m1, issuer))
                pending_stores = rest

            l2ps1 = set(map(tuple, plan.get("l2ps1", [])))
            ev_list = [tuple(x) for x in plan["l2ev"]]
            ei = 0

            def flush():
                nonlocal ei
                while ei < len(ev_list) and (ev_list[ei][0], ev_list[ei][1]) in mm_done:
                    ee, lo, eng, mode = ev_list[ei]
                    evict_l2(ee, lo, eng, mode)
                    banks_done[ee].update((lo, lo + 1))
                    ei += 1
                    emit_ready_stores()

            for e, mp in plan["l2ord"]:
                lo = 2 * mp
                if (e, lo) in l2ps1:
                    pp = ps1_pool.tile([128, 1024], f32, name="pp1")
                else:
                    pp = ps2_pool.tile([128, 1024], f32, name="pp2")
                l2_matmul(e, lo, pp[:, 0:C], hid_v[e])
                l2_matmul(e, lo + 1, pp[:, 512 : 512 + C], hid_v[e])
                pp2[(e, lo)] = pp
                mm_done.add((e, lo))
                flush()
            flush()

    nc.compile()
    return nc


def kernel(x, y_index, W_in, b_in, W_out, b_out):
    global LAST_RESULT
    from concourse.bass_utils import run_bass_kernel_spmd

    x = np.asarray(x, dtype=np.float32)
    W_in = np.asarray(W_in, dtype=np.float32)
    b_in = np.asarray(b_in, dtype=np.float32)
    W_out = np.asarray(W_out, dtype=np.float32)
    b_out = np.asarray(b_out, dtype=np.float32)
    eidx = np.asarray(y_index).reshape(-1).astype(np.int64)

    counts = np.bincount(eidx, minlength=NB)
    C = max(276, int(-(-counts.max() // 4) * 4))  # capacity per expert

    if C > 512:
        # Extreme expert skew would overflow a PSUM bank (512 f32 free dim);
        # fall back to exact host math rather than ship a broken program.
        out = np.empty_like(x)
        h_full = np.maximum(x, 0.0)
        for e in range(NB):
            m = eidx == e
            if m.any():
                hidv = h_full[m] @ W_in[e].T + b_in[e]
                out[m] = x[m] + hidv @ W_out[e].T + b_out[e]
        return out

    # --- host dispatch: group tokens by expert ---------------------------
    order = np.argsort(eidx, kind="stable")
    starts = np.zeros(NB + 1, dtype=np.int64)
    np.cumsum(counts, out=starts[1:])

    h = np.maximum(x, 0.0)
    Xg = np.zeros((NB, C, D), dtype=np.float32)
    for e in range(NB):
        toks = order[starts[e] : starts[e + 1]]
        Xg[e, : counts[e]] = h[toks]

    # Fold both biases into one host-side per-expert vector (exact f32).
    bML = np.einsum("eds,es->ed", W_out, b_in) + b_out  # [NB, D]

    # hT: [core, 128, (e, k, c)] - value = h[token (e,c), 128k + p]
    hT_all = np.ascontiguousarray(
        Xg.astype(F8)
        .reshape(NCORES, EPC, C, KD, 128)
        .transpose(0, 4, 1, 3, 2)
        .reshape(NCORES, 128, EPC * KD * C)
    )
    # w1: [core, 128, (e, t, k2, i, m)] = W_in[e, 128t+m, 256k2+128i+p] * 256
    w1_all = np.ascontiguousarray(
        (W_in * WSCALE)
        .astype(F8)
        .reshape(NCORES, EPC, KS, 128, K2, 2, 128)
        .transpose(0, 6, 1, 2, 4, 5, 3)
        .reshape(NCORES, 128, EPC * 2048)
    )
    # w2: [core, 128, (e, m, i, j)] = W_out[e, 128m+j, 128i+p] * 256
    w2_all = np.ascontiguousarray(
        (W_out * WSCALE)
        .astype(F8)
        .reshape(NCORES, EPC, KD, 128, KS, 128)
        .transpose(0, 5, 1, 2, 4, 3)
        .reshape(NCORES, 128, EPC * 2048)
    )

    if C not in _programs:
        _programs[C] = _build_program(C)
    nc = _programs[C]

    in_maps = [
        {"hT": hT_all[i], "w1": w1_all[i], "w2": w2_all[i]} for i in range(NCORES)
    ]
    res = run_bass_kernel_spmd(nc, in_maps, list(range(NCORES)))
    LAST_RESULT = res

    # --- host gather: decode fp8, add folded bias, scatter ---------------
    out = np.empty_like(x)
    Yg = np.stack(
        [np.asarray(r["yT"]).astype(np.float32) for r in res.results]
    )  # [NCORES, 128, EPC*KD*C]
    Yg = (
        Yg.reshape(NCORES, 128, EPC, KD, C)
        .transpose(0, 2, 4, 3, 1)
        .reshape(NB, C, D)
    )
    for e in range(NB):
        toks = order[starts[e] : starts[e + 1]]
        out[toks] = x[toks] + Yg[e, : counts[e]] + bML[e]
    return out
